# revision 1
# baseline (speedup 1.0000x reference)
"""Trainium2 Bass kernel for nn_Attention_update (additive attention pooling).

reference math (per example b):
    pre[s,d] = enc[b] @ W1e^T + (W1h @ h[b] + b1)      # [S, D]
    e[s]     = tanh(pre) @ W2[0]                        # [S]
    alpha    = softmax(e);  ctx = alpha @ enc[b]        # [DK]

Sharding: data-parallel over batch B=64 across 8 cores (8 examples/core),
same SPMD program on every core, no collectives.

Per-core kernel (score matmuls in float32r -> PE fast path, ~1 cyc/row,
~1e-4 matmul precision):
  - scores: per 128-row s-tile, PSUM chain over 8 k-chunks with stationary
    = transposed-enc tile [128k x 128s], moving = W1e^T [128k x 512d]
    -> pre in [s partitions, d free] layout.
  - bias row (W1h@h+b1, computed once at startup on PE) is broadcast to all
    128 partitions via a stride-0 DMA and added on VectorE while evacuating
    PSUM; ScalarE applies tanh.
  - e[s] = sum_d tanh*W2 in one fused VectorE scalar_tensor_tensor with
    accum_out -> e lands directly as [128, 16] (s on partitions), so the
    softmax needs no transposes.
  - p = exp(e) (no max subtraction: |e| is O(1) for this model); sum via a
    [128,1]x[128,1] ones-matmul; ctx = sum_t p[:,t] (x) encN-tile chains
    accumulated in PSUM, scaled by 1/sum at the end.  ctx of example b is
    emitted after the score phase of example b+1 so the PE never stalls on
    the softmax tail.
enc is supplied from host in both layouts (encT for scores, encN for
context): 2x DMA (~137 MB/core @ ~360 GB/s) hides fully under the
PE-bound ~550 us runtime.  Measured: rel err ~1e-4 vs fp32 reference,
~0.55 ms/call steady-state on 8 cores.
"""
import numpy as np

import concourse.bass as bass
import concourse.mybir as mybir
import concourse.tile as tile
from concourse import bacc
from concourse.bass import ts
from concourse.bass_utils import run_bass_kernel_spmd

AF = mybir.ActivationFunctionType
ALU = mybir.AluOpType
F32 = mybir.dt.float32
F32R = mybir.dt.float32r

N_CORES = 8
B, S, DK, D = 64, 2048, 1024, 1024
BC = B // N_CORES          # examples per core
KC = DK // 128             # k chunks
MC = D // 128              # m chunks (hidden dim)
NST = S // 128             # s-tiles per example
DH = 2                     # d halves (512 moving cols each)


def build_kernel(reps: int = 1, bc: int = BC, s: int = S, sdt=F32R):
    nst = s // 128
    nc = bacc.Bacc(None)

    encT = nc.dram_tensor("encT", [DK, bc * s], sdt, kind="ExternalInput")
    encN = nc.dram_tensor("encN", [bc * s, DK], F32R, kind="ExternalInput")
    w1eT = nc.dram_tensor("w1eT", [128, KC, D], sdt, kind="ExternalInput")
    w1hT = nc.dram_tensor("w1hT", [128, MC, D], F32R, kind="ExternalInput")
    hT = nc.dram_tensor("hT", [128, MC, bc], F32R, kind="ExternalInput")
    b1r = nc.dram_tensor("b1r", [1, D], F32R, kind="ExternalInput")
    w2r = nc.dram_tensor("w2r", [1, D], F32, kind="ExternalInput")
    out_d = nc.dram_tensor("out", [bc, DK], F32, kind="ExternalOutput")

    with tile.TileContext(nc) as tc:
        with (
            tc.tile_pool(name="consts", bufs=1) as consts,
            tc.tile_pool(name="smalls", bufs=4) as smalls,
            tc.tile_pool(name="prep", bufs=5, space="PSUM") as prep,
            tc.tile_pool(name="miscps", bufs=1, space="PSUM") as miscps,
            tc.tile_pool(name="sumps", bufs=1, space="PSUM") as sumps,
        ):
            # ---- constants / parameters ----
            w1eT_sb = consts.tile([128, KC, D], sdt)
            nc.sync.dma_start(out=w1eT_sb, in_=w1eT[:, :, :])
            # W2 broadcast to all 128 partitions
            w2b_sb = consts.tile([128, D], F32)
            w2_ap = w2r[0:1, :]
            nc.sync.dma_start(
                out=w2b_sb,
                in_=bass.AP(tensor=w2_ap.tensor, offset=w2_ap.offset,
                            ap=[[0, 128]] + list(w2_ap.ap[1:])),
            )
            ones_col = consts.tile([128, 1], F32)
            nc.vector.memset(ones_col, 1.0)
            hb8 = consts.tile([bc, D], F32)

            # ---- startup: hvec = W1h @ h + b1 for all bc examples.
            # W1h lives in a scoped pool released before the streaming pools
            # open (32 KB/partition that the steady state can't afford).
            with tc.tile_pool(name="w1hp", bufs=1) as w1hp:
                w1hT_sb = w1hp.tile([128, MC, D], F32R)
                nc.sync.dma_start(out=w1hT_sb, in_=w1hT[:, :, :])
                hT_sb = w1hp.tile([128, MC, bc], F32R)
                nc.sync.dma_start(out=hT_sb, in_=hT[:, :, :])
                b1b_sb = w1hp.tile([bc, D], F32R)
                b1_ap = b1r[0:1, :]
                nc.sync.dma_start(
                    out=b1b_sb,
                    in_=bass.AP(tensor=b1_ap.tensor, offset=b1_ap.offset,
                                ap=[[0, bc]] + list(b1_ap.ap[1:])),
                )
                hv_ps = miscps.tile([bc, D], F32, tag="misc")
                for mc in range(MC):
                    for dh in range(DH):
                        nc.tensor.matmul(
                            hv_ps[:, ts(dh, 512)],
                            hT_sb[:, mc, :], w1hT_sb[:, mc, ts(dh, 512)],
                            start=(mc == 0), stop=(mc == MC - 1),
                        )
                nc.vector.tensor_add(hb8, hv_ps, b1b_sb)

            with (
                tc.tile_pool(name="hbd_pool", bufs=1, space="DRAM") as hbdp,
                tc.tile_pool(name="encp", bufs=4) as encp,
                tc.tile_pool(name="encn", bufs=10) as encn,
                tc.tile_pool(name="tanhp", bufs=3) as tanhp,
                tc.tile_pool(name="ttrs", bufs=1) as ttrs,
                tc.tile_pool(name="ep", bufs=2) as ep,
                tc.tile_pool(name="hbp", bufs=2) as hbp,
                tc.tile_pool(name="biasp", bufs=3) as biasp,
                tc.tile_pool(name="outp", bufs=2) as outp,
            ):
              hbd = hbdp.tile([bc, D], F32)
              nc.sync.dma_start(out=hbd, in_=hb8[:, :])

              def body(_iv=None):
                # ---- per-example pipeline ----
                def scores_phase(b):
                    # this example's bias row -> partition 0
                    hb_bc = hbp.tile([128, D], F32)
                    hrow_ap = hbd[b:b + 1, :]
                    nc.sync.dma_start(
                        out=hb_bc,
                        in_=bass.AP(tensor=hrow_ap.tensor, offset=hrow_ap.offset,
                                    ap=[[0, 128]] + list(hrow_ap.ap[1:])))
                    e_sb = ep.tile([128, nst], F32, tag="e")
                    p_sb = ep.tile([128, nst], F32R, tag="p")
                    for h in range(s // 512):           # quarters of s
                        encTt = encp.tile([128, KC, 512], sdt)
                        e_ap = encT[:, :]
                        nc.sync.dma_start(
                            out=encTt,
                            in_=bass.AP(tensor=e_ap.tensor,
                                        offset=b * s + h * 512,
                                        ap=[[bc * s, 128], [128 * bc * s, KC],
                                            [1, 512]]),
                        )
                        for t4 in range(4):
                            t = h * 4 + t4
                            tanh_t = tanhp.tile([128, D], F32)
                            for dh in range(DH):
                                pre = prep.tile([128, 512], F32)
                                for kc in range(KC):
                                    nc.tensor.matmul(
                                        pre, encTt[:, kc, ts(t4, 128)],
                                        w1eT_sb[:, kc, ts(dh, 512)],
                                        start=(kc == 0), stop=(kc == KC - 1),
                                    )
                                biased = biasp.tile([128, 512], F32)
                                nc.vector.scalar_tensor_tensor(
                                    out=biased, in0=pre, scalar=0.0,
                                    in1=hb_bc[:, ts(dh, 512)],
                                    op0=ALU.add, op1=ALU.add)
                                nc.scalar.activation(
                                    tanh_t[:, ts(dh, 512)], biased, AF.Tanh)
                            ttr_o = ttrs.tile([128, D], F32, tag="ttr")
                            nc.vector.scalar_tensor_tensor(
                                out=ttr_o, in0=tanh_t, scalar=0.0,
                                in1=w2b_sb, op0=ALU.add, op1=ALU.mult,
                                accum_out=e_sb[:, t:t + 1],
                            )
                    nc.scalar.activation(p_sb, e_sb, AF.Exp)
                    pcs = smalls.tile([128, 1], F32, tag="pcs")
                    nc.vector.reduce_sum(pcs, p_sb, axis=mybir.AxisListType.X)
                    sum_ps = sumps.tile([1, 1], F32)
                    nc.tensor.matmul(sum_ps, pcs, ones_col, start=True, stop=True)
                    rs = smalls.tile([1, 1], F32, tag="rs")
                    nc.vector.reciprocal(rs, sum_ps)
                    return p_sb, rs

                def ctx_phase(b, p_sb, rs):
                    ctx_ps = miscps.tile([1, DK], F32, tag="misc")
                    for t in range(nst):
                        encNt = encn.tile([128, DK], F32R)
                        nc.sync.dma_start(
                            out=encNt,
                            in_=encN[b * s + t * 128: b * s + (t + 1) * 128, :],
                        )
                        for dh in range(DH):
                            nc.tensor.matmul(
                                ctx_ps[:, ts(dh, 512)],
                                p_sb[:, t:t + 1], encNt[:, ts(dh, 512)],
                                start=(t == 0), stop=(t == nst - 1),
                            )
                    ctx_sb = outp.tile([1, DK], F32)
                    nc.vector.tensor_scalar_mul(ctx_sb, ctx_ps, rs)
                    nc.sync.dma_start(out=out_d[b:b + 1, :], in_=ctx_sb)

                prev = None
                for b in range(bc):
                    cur = scores_phase(b)
                    if prev is not None:
                        ctx_phase(b - 1, *prev)
                    prev = cur
                ctx_phase(bc - 1, *prev)

              if reps == 1:
                  body()
              else:
                  with tc.For_i(0, reps, 1) as _i:
                      body(_i)

    nc.compile()
    return nc


def prep_inputs(hidden_state, encoder_outputs, W1, b1, W2, score_np=np.float32):
    """Split + relayout full inputs into per-core in_maps."""
    hidden_state = np.ascontiguousarray(hidden_state, dtype=np.float32)
    encoder_outputs = np.asarray(encoder_outputs, dtype=np.float32)
    W1 = np.asarray(W1, dtype=np.float32)
    b1 = np.asarray(b1, dtype=np.float32)
    W2 = np.asarray(W2, dtype=np.float32)

    W1e, W1h = W1[:, :DK], W1[:, DK:]
    # w1eT[kl, kc, d] = W1e[d, kc*128+kl]
    w1eT = np.ascontiguousarray(W1e.T.reshape(KC, 128, D).transpose(1, 0, 2))
    # w1hT[ml, mc, d] = W1h[d, mc*128+ml]
    w1hT = np.ascontiguousarray(W1h.T.reshape(MC, 128, D).transpose(1, 0, 2))
    b1r = np.ascontiguousarray(b1.reshape(1, D))
    w2r = np.ascontiguousarray(W2.reshape(1, D))

    in_maps = []
    for c in range(N_CORES):
        sl = slice(c * BC, (c + 1) * BC)
        enc_c = encoder_outputs[sl]                      # [BC, S, DK]
        encT = np.ascontiguousarray(
            enc_c.transpose(2, 0, 1).reshape(DK, BC * S))
        encN = np.ascontiguousarray(enc_c.reshape(BC * S, DK))
        h_c = hidden_state[sl]                           # [BC, D]
        hT = np.ascontiguousarray(h_c.T.reshape(MC, 128, BC).transpose(1, 0, 2))
        in_maps.append({
            "encT": encT.astype(score_np), "encN": encN,
            "w1eT": w1eT.astype(score_np), "w1hT": w1hT,
            "hT": hT, "b1r": b1r, "w2r": w2r,
        })
    return in_maps


_NC_CACHE = {}


def kernel(hidden_state, encoder_outputs, W1, b1, W2):
    if "nc" not in _NC_CACHE:
        _NC_CACHE["nc"] = build_kernel(reps=1)
    nc = _NC_CACHE["nc"]
    in_maps = prep_inputs(hidden_state, encoder_outputs, W1, b1, W2)
    res = run_bass_kernel_spmd(nc, in_maps, core_ids=list(range(N_CORES)))
    return np.concatenate([r["out"] for r in res.results], axis=0)



# revision 5
# speedup vs baseline: 2.5407x; 2.5407x over previous
"""Trainium2 Bass kernel for nn_Attention_update (additive attention pooling).

reference math (per example b):
    pre[s,d] = enc[b] @ W1e^T + (W1h @ h[b] + b1)      # [S, D]
    e[s]     = tanh(pre) @ W2[0]                        # [S]
    alpha    = softmax(e);  ctx = alpha @ enc[b]        # [DK]

Sharding: data-parallel over batch B=64 across 8 cores (8 examples/core),
same SPMD program on every core, no collectives.

Per-core kernel, fp8 DoubleRow score path in [d-part, s-free] layout:
  - scores: stationary = W1e chunks quantized to fp8e4 (pre-scaled x64 to
    dodge subnormal truncation), moving = enc in fp8e4 (pre-scaled x8),
    perf_mode=DoubleRow -> K=256 per instruction, ~2x bf16 MAC rate.
    pre lands as [128 d-part, 512 s-free] PSUM tiles (8 d-blocks x 4 j).
  - bias+tanh fused on ScalarE: tanh(pre/512 + hv[d]) with the per-example
    hv = W1h@h+b1 column as the ACT per-partition bias operand; output
    written bf16 straight into SBUF.
  - e[s] = sum_d W2_d tanh: K=128 matmul chain over the 8 d-blocks with
    W2 column stationary -> e as [1, 512] PSUM; a strided DMA transposes
    it into eT [128 s-part, 16]; ACT exp with accum_out gives p and its
    per-partition sum; ones-matmul + reciprocal finish the softmax scale.
  - ctx = sum_t p[:,t] (x) encN-tile chains in bf16, scaled by 1/sum.
    ctx of example b is emitted during the score phase of example b+1.
enc is supplied from host in fp8 (scores, 16.8MB) and bf16 (context,
33.5MB): ~50MB/core @ ~285GB/s hides under the PE-bound runtime.
Accuracy: fp8 quantization of enc/W1e gives rel err ~1e-2 on the fixed
harness inputs (vs 2e-2 gate); verified by numpy simulation + HW run.
"""
import numpy as np
import ml_dtypes

import concourse.bass as bass
import concourse.mybir as mybir
import concourse.tile as tile
from concourse import bacc
from concourse.bass import ts
from concourse.bass_utils import run_bass_kernel_spmd

AF = mybir.ActivationFunctionType
ALU = mybir.AluOpType
F32 = mybir.dt.float32
F32R = mybir.dt.float32r
BF16 = mybir.dt.bfloat16
FP8 = mybir.dt.float8e4
DRW = mybir.MatmulPerfMode.DoubleRow

N_CORES = 8
B, S, DK, D = 64, 2048, 1024, 1024
BC = B // N_CORES          # examples per core
MC = D // 128              # m chunks (hidden dim)
NDB = D // 128             # d blocks
NJ = DK // 256             # DoubleRow k chunks (K=256 each)
NT = S // 512              # s512 tiles per example
NST = S // 128             # s tiles for ctx
SW = 64.0                  # W1e pre-quantization scale
SE = 8.0                   # enc pre-quantization scale
SCALE = 1.0 / (SW * SE)    # descale folded into ACT


def build_kernel(reps: int = 1):
    nc = bacc.Bacc(None)

    encdr = nc.dram_tensor("encdr", [128, 2 * NJ, BC * S], FP8, kind="ExternalInput")
    encn = nc.dram_tensor("encn", [BC * S, DK], BF16, kind="ExternalInput")
    w1edr = nc.dram_tensor("w1edr", [128, 2 * NJ, D], FP8, kind="ExternalInput")
    w1hT = nc.dram_tensor("w1hT", [128, MC, D], F32R, kind="ExternalInput")
    hT = nc.dram_tensor("hT", [128, MC, BC], F32R, kind="ExternalInput")
    b1bc = nc.dram_tensor("b1bc", [128, NDB * BC], F32, kind="ExternalInput")
    w2dp = nc.dram_tensor("w2dp", [128, NDB], BF16, kind="ExternalInput")
    out_d = nc.dram_tensor("out", [BC, DK], F32, kind="ExternalOutput")

    with tile.TileContext(nc) as tc:
        with (
            tc.tile_pool(name="consts", bufs=1) as consts,
            tc.tile_pool(name="smalls", bufs=4) as smalls,
            tc.tile_pool(name="prep", bufs=4, space="PSUM") as prep,
            tc.tile_pool(name="epsp", bufs=1, space="PSUM") as epsp,
            tc.tile_pool(name="miscps", bufs=1, space="PSUM") as miscps,
            tc.tile_pool(name="sumps", bufs=1, space="PSUM") as sumps,
        ):
            # ---- constants / parameters ----
            w1e_sb = consts.tile([128, 2 * NJ, D], FP8)
            nc.sync.dma_start(out=w1e_sb, in_=w1edr[:, :, :])
            w2_sb = consts.tile([128, NDB], BF16)
            nc.sync.dma_start(out=w2_sb, in_=w2dp[:, :])
            ones_col = consts.tile([128, 1], F32)
            nc.vector.memset(ones_col, 1.0)
            hv_sb = consts.tile([128, NDB * BC], F32)

            # ---- startup: hv[d, b] = (W1h @ h + b1) in d-part layout ----
            with tc.tile_pool(name="w1hp", bufs=1) as w1hp:
                w1hT_sb = w1hp.tile([128, MC, D], F32R)
                nc.sync.dma_start(out=w1hT_sb, in_=w1hT[:, :, :])
                hT_sb = w1hp.tile([128, MC, BC], F32R)
                nc.sync.dma_start(out=hT_sb, in_=hT[:, :, :])
                b1_sb = w1hp.tile([128, NDB * BC], F32)
                nc.sync.dma_start(out=b1_sb, in_=b1bc[:, :])
                hv_ps = miscps.tile([128, NDB * BC], F32, tag="misc")
                for db in range(NDB):
                    sl = slice(db * BC, (db + 1) * BC)
                    for mc in range(MC):
                        nc.tensor.matmul(
                            hv_ps[:, sl], w1hT_sb[:, mc, ts(db, 128)],
                            hT_sb[:, mc, :], start=(mc == 0), stop=(mc == MC - 1))
                nc.vector.tensor_add(hv_sb, hv_ps, b1_sb)

            with (
                tc.tile_pool(name="edram", bufs=2, space="DRAM") as edram,
                tc.tile_pool(name="encp", bufs=4) as encp,
                tc.tile_pool(name="encnp", bufs=16) as encnp,
                tc.tile_pool(name="tanhp", bufs=2) as tanhp,
                tc.tile_pool(name="etp", bufs=2) as etp,
                tc.tile_pool(name="pbp", bufs=2) as pbp,
                tc.tile_pool(name="outp", bufs=2) as outp,
            ):
              def body(_iv=None):
                def scores_tile(b, t, tanh_t):
                    enct = encp.tile([128, 2 * NJ, 512], FP8)
                    nc.sync.dma_start(
                        out=enct,
                        in_=encdr[:, :, b * S + t * 512: b * S + (t + 1) * 512])
                    for dbp in range(NDB // 2):
                        pres = []
                        for h in range(2):
                            pre_t = prep.tile([128, 512], F32, tag="pre",
                                              name=f"pre{h}")
                            pres.append(pre_t)
                        for j in range(NJ):      # bank-interleaved DR chains
                            for h in range(2):
                                db = dbp * 2 + h
                                nc.tensor.matmul(
                                    pres[h],
                                    w1e_sb[:, 2 * j:2 * j + 2, ts(db, 128)],
                                    enct[:, 2 * j:2 * j + 2, :],
                                    start=(j == 0), stop=(j == NJ - 1),
                                    perf_mode=DRW)
                        for h in range(2):
                            db = dbp * 2 + h
                            nc.scalar.activation(
                                tanh_t[:, db], pres[h], AF.Tanh,
                                bias=hv_sb[:, db * BC + b: db * BC + b + 1],
                                scale=SCALE)

                def ered_tile(t, tanh_t, eT):
                    e_ps = epsp.tile([1, 512], F32)
                    for db in range(NDB):
                        nc.tensor.matmul(
                            e_ps, w2_sb[:, db:db + 1], tanh_t[:, db],
                            start=(db == 0), stop=(db == NDB - 1))
                    # evacuate PSUM, then transpose [1, 512] -> [128, 4] by
                    # bouncing through DRAM with a strided read-back
                    e_sb = smalls.tile([1, 512], F32, tag="esb")
                    nc.vector.tensor_copy(e_sb, e_ps)
                    e_d = edram.tile([1, 512], F32)
                    nc.sync.dma_start(out=e_d, in_=e_sb)
                    ed_ap = e_d[0:1, :]
                    nc.sync.dma_start(
                        out=eT[:, t * 4:(t + 1) * 4],
                        in_=bass.AP(tensor=ed_ap.tensor, offset=ed_ap.offset,
                                    ap=[[1, 128], [128, 4]]))

                def softmax(eT):
                    p_f = etp.tile([128, NST], F32, tag="pf")
                    pcs = smalls.tile([128, 1], F32, tag="pcs")
                    nc.scalar.activation(p_f, eT, AF.Exp, accum_out=pcs)
                    sum_ps = sumps.tile([1, 1], F32)
                    nc.tensor.matmul(sum_ps, pcs, ones_col, start=True, stop=True)
                    rs = smalls.tile([1, 1], F32, tag="rs")
                    nc.vector.reciprocal(rs, sum_ps)
                    p_bf = pbp.tile([128, NST], BF16)
                    nc.vector.tensor_copy(p_bf, p_f)
                    return p_bf, rs

                def ctx_phase(b, p_bf, rs):
                    ctx_ps = miscps.tile([1, DK], F32, tag="misc")
                    for st in range(NST):
                        encnt = encnp.tile([128, DK], BF16)
                        nc.sync.dma_start(
                            out=encnt,
                            in_=encn[b * S + st * 128: b * S + (st + 1) * 128, :])
                        for dh in range(2):
                            nc.tensor.matmul(
                                ctx_ps[:, ts(dh, 512)], p_bf[:, st:st + 1],
                                encnt[:, ts(dh, 512)],
                                start=(st == 0), stop=(st == NST - 1))
                    ctx_sb = outp.tile([1, DK], F32)
                    nc.vector.tensor_scalar_mul(ctx_sb, ctx_ps, rs)
                    nc.sync.dma_start(out=out_d[b:b + 1, :], in_=ctx_sb)

                prev = None
                for b in range(BC):
                    eT = etp.tile([128, NST], F32, tag="eT")
                    tanh_tiles = []
                    for t in range(NT):
                        tanh_t = tanhp.tile([128, NDB, 512], BF16)
                        scores_tile(b, t, tanh_t)
                        tanh_tiles.append(tanh_t)
                        if t >= 1:
                            ered_tile(t - 1, tanh_tiles[t - 1], eT)
                        if t == 2 and prev is not None:
                            ctx_phase(*prev)
                    ered_tile(NT - 1, tanh_tiles[NT - 1], eT)
                    p_bf, rs = softmax(eT)
                    prev = (b, p_bf, rs)
                ctx_phase(*prev)

              if reps == 1:
                  body()
              else:
                  with tc.For_i(0, reps, 1) as _i:
                      body(_i)

    nc.compile()
    return nc


def prep_inputs(hidden_state, encoder_outputs, W1, b1, W2):
    """Split + relayout + quantize full inputs into per-core in_maps."""
    E4, BF = ml_dtypes.float8_e4m3, ml_dtypes.bfloat16
    hidden_state = np.ascontiguousarray(hidden_state, dtype=np.float32)
    encoder_outputs = np.asarray(encoder_outputs, dtype=np.float32)
    W1 = np.asarray(W1, dtype=np.float32)
    b1 = np.asarray(b1, dtype=np.float32)
    W2 = np.asarray(W2, dtype=np.float32)

    W1e, W1h = W1[:, :DK], W1[:, DK:]
    # w1edr[p, 2j+i, d] = SW * W1e[d, j*256 + i*128 + p]
    w1edr = np.ascontiguousarray(
        (W1e.T * SW).reshape(NJ, 2, 128, D).transpose(2, 0, 1, 3)
        .reshape(128, 2 * NJ, D)).astype(E4)
    w1hT = np.ascontiguousarray(W1h.T.reshape(MC, 128, D).transpose(1, 0, 2))
    b1dp = b1.reshape(NDB, 128).T                     # [128, NDB]
    b1bc = np.ascontiguousarray(
        np.repeat(b1dp[:, :, None], BC, axis=2).reshape(128, NDB * BC),
        dtype=np.float32)
    w2dp = np.ascontiguousarray(W2.reshape(NDB, 128).T).astype(BF)

    in_maps = []
    for c in range(N_CORES):
        sl = slice(c * BC, (c + 1) * BC)
        enc_c = encoder_outputs[sl]                      # [BC, S, DK]
        # encdr[p, 2j+i, b*S+s] = SE * enc[b, s, j*256 + i*128 + p]
        encdr = np.ascontiguousarray(
            (enc_c.transpose(2, 0, 1).reshape(NJ, 2, 128, BC * S) * SE)
            .transpose(2, 0, 1, 3).reshape(128, 2 * NJ, BC * S)).astype(E4)
        encn = np.ascontiguousarray(enc_c.reshape(BC * S, DK)).astype(BF)
        h_c = hidden_state[sl]                           # [BC, D]
        hT = np.ascontiguousarray(h_c.T.reshape(MC, 128, BC).transpose(1, 0, 2))
        in_maps.append({
            "encdr": encdr, "encn": encn, "w1edr": w1edr,
            "w1hT": w1hT, "hT": hT, "b1bc": b1bc, "w2dp": w2dp,
        })
    return in_maps


_NC_CACHE = {}


def kernel(hidden_state, encoder_outputs, W1, b1, W2):
    if "nc" not in _NC_CACHE:
        _NC_CACHE["nc"] = build_kernel(reps=1)
    nc = _NC_CACHE["nc"]
    in_maps = prep_inputs(hidden_state, encoder_outputs, W1, b1, W2)
    res = run_bass_kernel_spmd(nc, in_maps, core_ids=list(range(N_CORES)))
    return np.concatenate([r["out"] for r in res.results], axis=0)
